# revision 21
# baseline (speedup 1.0000x reference)
"""Bahdanau attention Trainium2 Bass kernel.

Problem: B=128, T=2048, ENC_H=DEC_H=512, ATTN=128.
    enc_proj = einsum("bth,ah->bta", enc, W_enc)
    dec_proj = einsum("bh,ah->ba", dec, W_dec)
    energy   = tanh(enc_proj + dec_proj[:, None, :])
    scores   = einsum("bta,a->bt", energy, v)
    weights  = softmax(scores, -1)
    context  = einsum("bt,bth->bh", weights, enc)
    returns (context [B,512], weights [B,2048])

Sharding: data-parallel over B across 8 NeuronCores (16 batches/core);
projection weights replicated. Tiny derived constants (W_enc^T in SBUF
layout, dec_proj = W_dec @ dec^T, v as a column, identity, ones) are
precomputed on the host and shipped as extra inputs — this removes all
device-side setup work. The softmax normalization (divide by the exp-sum)
is also done on the host: the device returns unnormalized exp-weights,
unnormalized context, and the per-batch exp sums.

Per-core device pipeline (per batch b, enc[b] read from HBM exactly once):
  1. enc[b] -> SBUF native [t-part, h-free], resident (16 t-tiles).
  2. PE transpose-mode matmuls turn 128x128 blocks into encT [h-part, t]
     (PSUM), evacuated to SBUF by ACT/DVE (engine fixed per psum slot so
     slot-release waits stay subsumed — walrus allows only ONE sync-wait
     on transpose-mode matmuls).
  3. proj: psum[a, t512] = sum_c W_encT[c].T @ encT[c]  (regular matmuls).
  4. ACT tanh with per-partition bias dec_projT[:, b] -> SBUF [a, t].
  5. scores emitted directly transposed: lhsT=tanh[:, t128], rhs=v_col
     -> psum scb [t128-part, 16].
  6. ACT exp(scb) with accum_out -> exp_sb [128,16] + per-partition sums;
     PE ones-matmul reduces sums across partitions -> S (psum [1,1]).
  7. weights out: PE transpose exp_sb -> [16,128] rows; context: 16
     accumulating M=1 matmuls lhsT=exp[:, j], rhs=enc_nat[j] (N=512).
  8. DVE stages everything into resident output tiles; 3 DMAs at the end.
"""

import os

os.environ.setdefault("MYCRO_LOCAL_CACHE", "1")

import sys
from contextlib import ExitStack

import numpy as np

if "/opt/trn_rl_repo" not in sys.path:
    sys.path.insert(0, "/opt/trn_rl_repo")

import concourse.bass as bass
import concourse.mybir as mybir
import concourse.tile as tile
from concourse import bass_utils
from concourse.bass import ts

F32 = mybir.dt.float32
F32R = mybir.dt.float32r

B, T, H, A = 128, 2048, 512, 128
NCORES = 8
BL = B // NCORES  # 16 batches per core
P = 128
HC = H // P  # 4 h-chunks
NT = T // P  # 16 t-tiles per batch
TCH = 512  # proj free-dim chunk
NTC = T // TCH  # 4 chunks per batch
JPC = TCH // P  # 4 t-tiles per chunk

# knobs (env-overridable for experiments)
MM_DT = {"f32": F32, "f32r": F32R}[os.environ.get("BAH_MM_DT", "f32")]
TR_DT = {"f32": F32, "f32r": F32R}[os.environ.get("BAH_TR_DT", "f32")]
CTX_DT = {"f32": F32, "f32r": F32R}[os.environ.get("BAH_CTX_DT", "f32")]
# of every 4 tr-psum slots, how many evacuate via ACT (rest DVE)
ACT_SLOTS = int(os.environ.get("BAH_ACT_SLOTS", "2"))


def _bc(ap, dt_):
    return ap.bitcast(dt_) if dt_ != ap.dtype else ap


def build_kernel(ctx: ExitStack, tc: tile.TileContext, enc, wencT_d, dec_proj_d,
                 v_col_d, ones_d, ident_d, wts_out, ctx_out, s_out):
    nc = tc.nc
    Tanh = mybir.ActivationFunctionType.Tanh
    Exp = mybir.ActivationFunctionType.Exp

    const = ctx.enter_context(tc.tile_pool(name="const", bufs=1))
    sb = ctx.enter_context(tc.tile_pool(name="sb", bufs=2))
    encp = ctx.enter_context(tc.tile_pool(name="encp", bufs=2))
    # PSUM (8 banks): tr(4) + pj(1) + scb/wrow/ctx shared tag(2) + smA(1)
    tr_ps = ctx.enter_context(tc.tile_pool(name="tr_ps", bufs=4, space="PSUM"))
    pj_ps = ctx.enter_context(tc.tile_pool(name="pj_ps", bufs=1, space="PSUM"))
    sc_ps = ctx.enter_context(tc.tile_pool(name="sc_ps", bufs=2, space="PSUM"))
    sm_ps = ctx.enter_context(tc.tile_pool(name="sm_ps", bufs=1, space="PSUM"))

    # ---------------- host-precomputed constants ----------------
    ident = const.tile([P, P], F32, name="ident")
    nc.sync.dma_start(ident, ident_d[:, :])
    wencT = const.tile([P, HC, A], F32, name="wencT")
    nc.sync.dma_start(wencT, wencT_d[:, :].rearrange("p (c a) -> p c a", a=A))
    dec_proj = const.tile([A, BL], F32, name="dec_proj")
    nc.sync.dma_start(dec_proj, dec_proj_d[:, :])
    v_col = const.tile([P, 1], F32, name="v_col")
    nc.sync.dma_start(v_col, v_col_d[:, :])
    ones_col = const.tile([P, 1], F32, name="ones_col")
    nc.sync.dma_start(ones_col, ones_d[:, :])

    # resident output staging (DVE-written, DMA'd out once at the end)
    w_all = const.tile([NT, BL, P], F32, name="w_all")
    ctx_all = const.tile([1, BL, H], F32, name="ctx_all")
    s_all = const.tile([1, BL], F32, name="s_all")

    # warmups: absorb every const-DMA queue semaphore into the consuming
    # engine's observed clock (walrus allows only ONE sync-wait per
    # instruction, so no later instruction may need two fresh semaphores).
    warm_ps = sm_ps.tile([1, P], F32, name="warm_ps", tag="smA")
    nc.tensor.matmul(warm_ps[:, 0:P], ident[:, 0:1], ident, start=True, stop=True)
    for wi, wsrc in enumerate((wencT[:, 0, 0:1], v_col, ones_col)):
        nc.tensor.matmul(warm_ps[:, wi:wi + 1], wsrc, ident[:, 0:1],
                         start=True, stop=True)
    warm_sb = sb.tile([A, BL], F32, name="warm_sb", tag="warm", bufs=1)
    nc.scalar.copy(warm_sb, dec_proj)

    tr_alloc = 0  # tr-pool alloc counter -> psum slot = tr_alloc % 4

    # ---------------- main loop over local batches ----------------
    for b in range(BL):
        enc_b = encp.tile([P, NT, H], F32, name="enc_b", tag="enc_b")
        for j in range(NT):
            nc.sync.dma_start(enc_b[:, j, :], enc[b, ts(j, P), :])

        # scores in transposed layout: [t128-part, j]
        scb = sc_ps.tile([P, NT], F32, name="scb", tag="scb")

        for t in range(NTC):
            encT_tiles = []
            for c in range(HC):
                trp = tr_ps.tile([P, TCH], F32, name="trp", tag="tr")
                slot = tr_alloc % 4
                tr_alloc += 1
                for jj in range(JPC):
                    j = t * JPC + jj
                    nc.tensor.transpose(
                        _bc(trp[:, ts(jj, P)], TR_DT),
                        _bc(enc_b[:, j, ts(c, P)], TR_DT),
                        _bc(ident, TR_DT),
                    )
                encT = sb.tile([P, TCH], F32, name="encT", tag="encT", bufs=4)
                # evac engine fixed per psum slot -> slot-release waits are
                # subsumed by the previous proj matmul's wait on that engine
                if slot < ACT_SLOTS:
                    nc.scalar.copy(encT, trp)
                else:
                    nc.vector.tensor_copy(encT, trp)
                encT_tiles.append(encT)

            pj = pj_ps.tile([A, TCH], F32, name="pj", tag="pj")
            for c in range(HC):
                nc.tensor.matmul(pj, _bc(wencT[:, c, :], MM_DT),
                                 _bc(encT_tiles[c], MM_DT),
                                 start=(c == 0), stop=(c == HC - 1))

            tanh_sb = sb.tile([A, TCH], F32, name="tanh_sb", tag="tanh", bufs=3)
            nc.scalar.activation(tanh_sb, pj, Tanh, bias=dec_proj[:, b:b + 1])

            for jj in range(JPC):
                j = t * JPC + jj
                nc.tensor.matmul(scb[:, j:j + 1],
                                 _bc(tanh_sb[:, ts(jj, P)], MM_DT),
                                 _bc(v_col, MM_DT), start=True, stop=True)

        # -------- softmax pieces (normalization happens on the host) -----
        exp_sb = sb.tile([P, NT], F32, name="exp_sb", tag="exp", bufs=2)
        sums = sb.tile([P, 1], F32, name="sums", tag="sums", bufs=2)
        nc.scalar.activation(exp_sb, scb, Exp, accum_out=sums)
        tot_ps = sm_ps.tile([1, 1], F32, name="tot_ps", tag="smA")
        nc.tensor.matmul(tot_ps, sums, ones_col, start=True, stop=True)
        nc.vector.tensor_copy(s_all[:, b:b + 1], tot_ps)

        # unnormalized weight rows: transpose exp [128,16] -> [16,128]
        wrow_ps = sc_ps.tile([BL, P], F32, name="wrow_ps", tag="scb")
        nc.tensor.transpose(wrow_ps, exp_sb, ident)
        nc.vector.tensor_copy(w_all[:, b, :], wrow_ps)

        # context: ctx[1, 512] = sum_j exp[:, j].T @ enc_b[:, j, :]
        ctx_ps = sc_ps.tile([1, H], F32, name="ctx_ps", tag="scb")
        for j in range(NT):
            nc.tensor.matmul(ctx_ps, _bc(exp_sb[:, j:j + 1], CTX_DT),
                             _bc(enc_b[:, j, :], CTX_DT),
                             start=(j == 0), stop=(j == NT - 1))
        nc.vector.tensor_copy(ctx_all[:, b, :], ctx_ps)

    # ---------------- final output DMAs ----------------
    nc.sync.dma_start(wts_out[:, :].rearrange("b (j q) -> j b q", q=P), w_all)
    nc.sync.dma_start(ctx_out[:, :].rearrange("b h -> (b h)")[None, :],
                      ctx_all.rearrange("o b h -> o (b h)"))
    nc.sync.dma_start(s_out[:, :], s_all)


def build_kernel_bf16(ctx: ExitStack, tc: tile.TileContext, enc, wencT_d,
                      dec_proj_d, v_col_d, ones_d, ident_d, wts_out, ctx_out,
                      s_out):
    """bf16 data path, no DRAM bounce: enc fp32 is cast to bf16 on DVE;
    128x128 PE transpose-mode matmuls (1 cycle/row in bf16) build encT in
    PSUM (bf16), evacuated to SBUF by ACT/DVE (bf16 = faster copies);
    proj/scores/ctx matmuls all run in bf16 (1 cycle/row vs 4 for fp32).
    Trades ~2e-3 relative error for the fp32 double-pass + fp32-transpose
    PE time.
    """
    nc = tc.nc
    BF16 = mybir.dt.float16 if os.environ.get("BAH_16", "fp16") == "fp16" \
        else mybir.dt.bfloat16
    Tanh = mybir.ActivationFunctionType.Tanh
    Exp = mybir.ActivationFunctionType.Exp

    const = ctx.enter_context(tc.tile_pool(name="const", bufs=1))
    sb = ctx.enter_context(tc.tile_pool(name="sb", bufs=2))
    encp = ctx.enter_context(tc.tile_pool(name="encp", bufs=2))
    tr_ps = ctx.enter_context(tc.tile_pool(name="tr_ps", bufs=4, space="PSUM"))
    pj_ps = ctx.enter_context(tc.tile_pool(name="pj_ps", bufs=1, space="PSUM"))
    sc_ps = ctx.enter_context(tc.tile_pool(name="sc_ps", bufs=2, space="PSUM"))
    sm_ps = ctx.enter_context(tc.tile_pool(name="sm_ps", bufs=1, space="PSUM"))

    # constants (host-precomputed)
    identf = const.tile([P, P], F32, name="identf")
    nc.sync.dma_start(identf, ident_d[:, :])
    ident = const.tile([P, P], BF16, name="ident")
    nc.vector.tensor_copy(ident, identf)
    wencT = const.tile([P, HC, A], BF16, name="wencT")
    nc.sync.dma_start(wencT, wencT_d[:, :].rearrange("p (c a) -> p c a", a=A))
    dec_proj = const.tile([A, BL], F32, name="dec_proj")
    nc.sync.dma_start(dec_proj, dec_proj_d[:, :])
    v_colf = const.tile([P, 1], F32, name="v_colf")
    nc.sync.dma_start(v_colf, v_col_d[:, :])
    v_col = const.tile([P, 1], BF16, name="v_col")
    nc.vector.tensor_copy(v_col, v_colf)
    ones_col = const.tile([P, 1], F32, name="ones_col")
    nc.sync.dma_start(ones_col, ones_d[:, :])

    # resident output staging
    w_all = const.tile([NT, BL, P], F32, name="w_all")
    ctx_all = const.tile([1, BL, H], F32, name="ctx_all")
    s_all = const.tile([1, BL], F32, name="s_all")

    # warmups: absorb const-DMA queue sems into consuming engines' clocks
    warm_ps = sm_ps.tile([1, P], F32, name="warm_ps", tag="smA")
    nc.tensor.matmul(warm_ps, ones_col, identf, start=True, stop=True)
    nc.tensor.matmul(warm_ps[:, 0:1], wencT[:, 0, 0:1], v_col, start=True,
                     stop=True)
    warm_sb = sb.tile([A, BL], F32, name="warm_sb", tag="warm", bufs=1)
    nc.scalar.copy(warm_sb, dec_proj)

    NLD = 8          # enc fp32 loads per batch ([128, 2, 512] each)
    JPL = NT // NLD  # 2 j-tiles per load

    GRP = 4  # context col-tiling group size
    tr_alloc = 0
    exp16s = []
    enc16s = []
    for b in range(BL):
        enc16 = encp.tile([P, NT, H], BF16, name="enc16", tag="enc16",
                          bufs=GRP + 2)

        # load fp32; cast to 16-bit split between DVE and (idle) GpSimd so
        # the casts clear quickly after their DMA lands
        for l in range(NLD):
            enc_f = sb.tile([P, JPL, H], F32, name="enc_f", tag="encf", bufs=6)
            nc.sync.dma_start(enc_f, enc[b, ts(l, JPL * P), :]
                              .rearrange("(j p) h -> p j h", p=P))
            if l % 2 == 0:
                nc.vector.tensor_copy(enc16[:, ts(l, JPL), :], enc_f)
            else:
                nc.gpsimd.tensor_copy(enc16[:, ts(l, JPL), :], enc_f)

        scb = sc_ps.tile([P, NT], F32, name="scb", tag="scb")

        for t in range(NTC):
            encT_tiles = []
            for c in range(HC):
                trp = tr_ps.tile([P, TCH], BF16, name="trp", tag="tr")
                slot = tr_alloc % 4
                tr_alloc += 1
                for jj in range(JPC):
                    j = t * JPC + jj
                    nc.tensor.transpose(trp[:, ts(jj, P)],
                                        enc16[:, j, ts(c, P)], ident)
                encT = sb.tile([P, TCH], BF16, name="encT", tag="encT", bufs=4)
                if slot < ACT_SLOTS:
                    nc.scalar.copy(encT, trp)
                else:
                    nc.vector.tensor_copy(encT, trp)
                encT_tiles.append(encT)

            pj = pj_ps.tile([A, TCH], F32, name="pj", tag="pj")
            for c in range(HC):
                nc.tensor.matmul(pj, wencT[:, c, :], encT_tiles[c],
                                 start=(c == 0), stop=(c == HC - 1))
            tanh16 = sb.tile([A, TCH], BF16, name="tanh16", tag="tanh", bufs=3)
            nc.scalar.activation(tanh16, pj, Tanh, bias=dec_proj[:, b:b + 1])
            for jj in range(JPC):
                j = t * JPC + jj
                nc.tensor.matmul(scb[:, j:j + 1], tanh16[:, ts(jj, P)],
                                 v_col, start=True, stop=True)

        # softmax pieces (normalization on host)
        exp_sb = sb.tile([P, NT], F32, name="exp_sb", tag="exp", bufs=2)
        sums = sb.tile([P, 1], F32, name="sums", tag="sums", bufs=2)
        nc.scalar.activation(exp_sb, scb, Exp, accum_out=sums)
        exp16 = sb.tile([P, NT], BF16, name="exp16", tag="exp16", bufs=GRP + 2)
        nc.gpsimd.tensor_copy(exp16, exp_sb)
        exp16s.append(exp16)
        enc16s.append(enc16)
        tot_ps = sm_ps.tile([1, 1], F32, name="tot_ps", tag="smA")
        nc.tensor.matmul(tot_ps, sums, ones_col, start=True, stop=True)
        # critical-path psum evacuations go to ACT: DVE's queue runs ~10us
        # behind (bulk casts of future batches), and these copies gate psum
        # slot reuse for the next batch's scores/ctx matmuls
        nc.scalar.copy(s_all[:, b:b + 1], tot_ps)

        wrow_ps = sc_ps.tile([BL, P], F32, name="wrow_ps", tag="scb")
        nc.tensor.transpose(wrow_ps, exp_sb, identf)
        nc.scalar.copy(w_all[:, b, :], wrow_ps)

        # context for a group of GRP batches at once: 4 concurrent M=1
        # matmuls in distinct 32-column groups of the PE array
        if b % GRP == GRP - 1:
            ctx_ps = sc_ps.tile([P, H], F32, name="ctx_ps", tag="scb")
            for j in range(NT):
                for gi in range(GRP):
                    nc.tensor.matmul(
                        ctx_ps[32 * gi:32 * gi + 1, :],
                        exp16s[gi][:, j:j + 1], enc16s[gi][:, j, :],
                        start=(j == 0), stop=(j == NT - 1),
                        tile_position=(0, 32 * gi))
            for gi in range(GRP):
                nc.scalar.copy(ctx_all[:, b - GRP + 1 + gi, :],
                               ctx_ps[32 * gi:32 * gi + 1, :])
            exp16s = []
            enc16s = []

    nc.sync.dma_start(wts_out[:, :].rearrange("b (j q) -> j b q", q=P), w_all)
    nc.sync.dma_start(ctx_out[:, :].rearrange("b h -> (b h)")[None, :],
                      ctx_all.rearrange("o b h -> o (b h)"))
    nc.sync.dma_start(s_out[:, :], s_all)


def _split_multi_waits(nc):
    """Walrus in this toolchain allows exactly ONE sync-wait per engine
    instruction. Tile sometimes emits more (slot-release + data deps on
    different semaphores). Hoist extra waits onto injected same-engine
    NoOps placed immediately before the instruction — semantically
    identical (engine streams are in-order), costs ~a sequencer dispatch.
    """
    n_split = 0
    for fn in nc.m.functions:
        for blk in fn.blocks:
            ins_list = blk.instructions
            if not any(i.sync_info and i.sync_info.on_wait and
                       len(i.sync_info.on_wait) > 1 for i in ins_list):
                continue
            out = []
            for inst in ins_list:
                si = inst.sync_info
                waits = list(si.on_wait) if si is not None and si.on_wait else []
                if len(waits) > 1:
                    # same-semaphore duplicates: keep only the max tick
                    bysem = {}
                    for w in waits:
                        k = (w.sync_type, w.id)
                        prev = bysem.get(k)
                        if prev is None or (w.wait_value or 0) > (prev.wait_value or 0):
                            bysem[k] = w
                    waits = list(bysem.values())
                if len(waits) > 1:
                    for w in waits[:-1]:
                        nop = mybir.InstNoOp(name=f"{inst.name}-ws{n_split}",
                                             engine=inst.engine, ins=[], outs=[])
                        nop.sync_info = mybir.SyncInfo(on_wait=[w], on_update=[])
                        out.append(nop)
                        n_split += 1
                    si.on_wait = [waits[-1]]
                out.append(inst)
            ins_list[:] = out
    return n_split


MODE = os.environ.get("BAH_MODE", "bf16")  # "bf16" | "f32"


def build_nc():
    nc = bass.Bass(trn_type="TRN2")
    if MODE == "bf16":
        wdt = (mybir.dt.float16 if os.environ.get("BAH_16", "fp16") == "fp16"
               else mybir.dt.bfloat16)
    else:
        wdt = F32
    enc = nc.dram_tensor("enc", [BL, T, H], F32, kind="ExternalInput")
    wencT_d = nc.dram_tensor("wencT", [P, HC * A], wdt, kind="ExternalInput")
    dec_proj_d = nc.dram_tensor("dec_projT", [A, BL], F32, kind="ExternalInput")
    v_col_d = nc.dram_tensor("v_col", [P, 1], F32, kind="ExternalInput")
    ones_d = nc.dram_tensor("ones_col", [P, 1], F32, kind="ExternalInput")
    ident_d = nc.dram_tensor("ident", [P, P], F32, kind="ExternalInput")
    wts_out = nc.dram_tensor("wts_raw", [BL, T], F32, kind="ExternalOutput")
    ctx_out = nc.dram_tensor("ctx_raw", [BL, H], F32, kind="ExternalOutput")
    s_out = nc.dram_tensor("s_out", [1, BL], F32, kind="ExternalOutput")

    build = build_kernel_bf16 if MODE == "bf16" else build_kernel
    with tile.TileContext(nc) as tc:
        with ExitStack() as ctx:
            build(ctx, tc, enc, wencT_d, dec_proj_d, v_col_d, ones_d,
                  ident_d, wts_out, ctx_out, s_out)
    _split_multi_waits(nc)
    return nc


_CACHE = {}


def _get_nc():
    if "nc" not in _CACHE:
        _CACHE["nc"] = build_nc()
    return _CACHE["nc"]


def _in_maps(encoder_outputs, decoder_hidden, W_enc, W_dec, v_w):
    enc = np.ascontiguousarray(np.asarray(encoder_outputs, dtype=np.float32))
    dec = np.ascontiguousarray(np.asarray(decoder_hidden, dtype=np.float32))
    we = np.asarray(W_enc, dtype=np.float32)
    wd = np.asarray(W_dec, dtype=np.float32)
    vw = np.asarray(v_w, dtype=np.float32)

    # host-derived constants (replicated)
    wencT = np.ascontiguousarray(
        we.T.reshape(HC, P, A).transpose(1, 0, 2).reshape(P, HC * A))
    v_col = np.ascontiguousarray(vw.reshape(A, 1))
    if MODE == "bf16":
        if os.environ.get("BAH_16", "fp16") == "fp16":
            wencT = wencT.astype(np.float16)
        else:
            import ml_dtypes
            wencT = wencT.astype(ml_dtypes.bfloat16)
    ones = np.ones((P, 1), dtype=np.float32)
    ident = np.eye(P, dtype=np.float32)

    maps = []
    for i in range(NCORES):
        sl = slice(i * BL, (i + 1) * BL)
        dec_projT = np.ascontiguousarray(wd @ dec[sl].T)  # [A, BL]
        maps.append({
            "enc": np.ascontiguousarray(enc[sl]),
            "wencT": wencT, "dec_projT": dec_projT, "v_col": v_col,
            "ones_col": ones, "ident": ident,
        })
    return maps


def run(encoder_outputs, decoder_hidden, W_enc, W_dec, v_w, **run_kwargs):
    """Run on 8 cores; returns (BassKernelResults, context, weights)."""
    nc = _get_nc()
    maps = _in_maps(encoder_outputs, decoder_hidden, W_enc, W_dec, v_w)
    res = bass_utils.run_bass_kernel_spmd(nc, maps, core_ids=list(range(NCORES)),
                                          **run_kwargs)
    ctxs = []
    wtss = []
    for r in res.results:
        s = r["s_out"].reshape(BL, 1)  # per-batch exp sums
        ctxs.append(r["ctx_raw"] / s)
        wtss.append(r["wts_raw"] / s)
    return res, np.concatenate(ctxs, axis=0), np.concatenate(wtss, axis=0)


def kernel(encoder_outputs, decoder_hidden, W_enc, W_dec, v_w):
    _, ctx, wts = run(encoder_outputs, decoder_hidden, W_enc, W_dec, v_w)
    return ctx, wts


if __name__ == "__main__":
    nc = _get_nc()
    print("built ok")
    if os.environ.get("BAH_COMPILE", "0") == "1":
        import tempfile
        d = tempfile.mkdtemp(prefix="bahcompile")
        print("compiling to", d)
        print("NEFF:", bass_utils.compile_bass_kernel(nc, d))


# revision 22
# speedup vs baseline: 1.1955x; 1.1955x over previous
"""Bahdanau attention Trainium2 Bass kernel.

Problem: B=128, T=2048, ENC_H=DEC_H=512, ATTN=128.
    enc_proj = einsum("bth,ah->bta", enc, W_enc)
    dec_proj = einsum("bh,ah->ba", dec, W_dec)
    energy   = tanh(enc_proj + dec_proj[:, None, :])
    scores   = einsum("bta,a->bt", energy, v)
    weights  = softmax(scores, -1)
    context  = einsum("bt,bth->bh", weights, enc)
    returns (context [B,512], weights [B,2048])

Sharding: data-parallel over B across 8 NeuronCores (16 batches/core);
projection weights replicated. Tiny derived constants (W_enc^T in SBUF
layout, dec_proj = W_dec @ dec^T, v as a column, identity, ones) are
precomputed on the host and shipped as extra inputs — this removes all
device-side setup work. The softmax normalization (divide by the exp-sum)
is also done on the host: the device returns unnormalized exp-weights,
unnormalized context, and the per-batch exp sums.

Per-core device pipeline (per batch b, enc[b] read from HBM exactly once):
  1. enc[b] -> SBUF native [t-part, h-free], resident (16 t-tiles).
  2. PE transpose-mode matmuls turn 128x128 blocks into encT [h-part, t]
     (PSUM), evacuated to SBUF by ACT/DVE (engine fixed per psum slot so
     slot-release waits stay subsumed — walrus allows only ONE sync-wait
     on transpose-mode matmuls).
  3. proj: psum[a, t512] = sum_c W_encT[c].T @ encT[c]  (regular matmuls).
  4. ACT tanh with per-partition bias dec_projT[:, b] -> SBUF [a, t].
  5. scores emitted directly transposed: lhsT=tanh[:, t128], rhs=v_col
     -> psum scb [t128-part, 16].
  6. ACT exp(scb) with accum_out -> exp_sb [128,16] + per-partition sums;
     PE ones-matmul reduces sums across partitions -> S (psum [1,1]).
  7. weights out: PE transpose exp_sb -> [16,128] rows; context: 16
     accumulating M=1 matmuls lhsT=exp[:, j], rhs=enc_nat[j] (N=512).
  8. DVE stages everything into resident output tiles; 3 DMAs at the end.
"""

import os

os.environ.setdefault("MYCRO_LOCAL_CACHE", "1")

import sys
from contextlib import ExitStack

import numpy as np

if "/opt/trn_rl_repo" not in sys.path:
    sys.path.insert(0, "/opt/trn_rl_repo")

import concourse.bass as bass
import concourse.mybir as mybir
import concourse.tile as tile
from concourse import bass_utils
from concourse.bass import ts

F32 = mybir.dt.float32
F32R = mybir.dt.float32r

B, T, H, A = 128, 2048, 512, 128
NCORES = 8
BL = B // NCORES  # 16 batches per core
P = 128
HC = H // P  # 4 h-chunks
NT = T // P  # 16 t-tiles per batch
TCH = 512  # proj free-dim chunk
NTC = T // TCH  # 4 chunks per batch
JPC = TCH // P  # 4 t-tiles per chunk

# knobs (env-overridable for experiments)
MM_DT = {"f32": F32, "f32r": F32R}[os.environ.get("BAH_MM_DT", "f32")]
TR_DT = {"f32": F32, "f32r": F32R}[os.environ.get("BAH_TR_DT", "f32")]
CTX_DT = {"f32": F32, "f32r": F32R}[os.environ.get("BAH_CTX_DT", "f32")]
# of every 4 tr-psum slots, how many evacuate via ACT (rest DVE)
ACT_SLOTS = int(os.environ.get("BAH_ACT_SLOTS", "2"))


def _bc(ap, dt_):
    return ap.bitcast(dt_) if dt_ != ap.dtype else ap


def build_kernel(ctx: ExitStack, tc: tile.TileContext, enc, wencT_d, dec_proj_d,
                 v_col_d, ones_d, ident_d, wts_out, ctx_out, s_out):
    nc = tc.nc
    Tanh = mybir.ActivationFunctionType.Tanh
    Exp = mybir.ActivationFunctionType.Exp

    const = ctx.enter_context(tc.tile_pool(name="const", bufs=1))
    sb = ctx.enter_context(tc.tile_pool(name="sb", bufs=2))
    encp = ctx.enter_context(tc.tile_pool(name="encp", bufs=2))
    # PSUM (8 banks): tr(4) + pj(1) + scb/wrow/ctx shared tag(2) + smA(1)
    tr_ps = ctx.enter_context(tc.tile_pool(name="tr_ps", bufs=4, space="PSUM"))
    pj_ps = ctx.enter_context(tc.tile_pool(name="pj_ps", bufs=1, space="PSUM"))
    sc_ps = ctx.enter_context(tc.tile_pool(name="sc_ps", bufs=2, space="PSUM"))
    sm_ps = ctx.enter_context(tc.tile_pool(name="sm_ps", bufs=1, space="PSUM"))

    # ---------------- host-precomputed constants ----------------
    ident = const.tile([P, P], F32, name="ident")
    nc.sync.dma_start(ident, ident_d[:, :])
    wencT = const.tile([P, HC, A], F32, name="wencT")
    nc.sync.dma_start(wencT, wencT_d[:, :].rearrange("p (c a) -> p c a", a=A))
    dec_proj = const.tile([A, BL], F32, name="dec_proj")
    nc.sync.dma_start(dec_proj, dec_proj_d[:, :])
    v_col = const.tile([P, 1], F32, name="v_col")
    nc.sync.dma_start(v_col, v_col_d[:, :])
    ones_col = const.tile([P, 1], F32, name="ones_col")
    nc.sync.dma_start(ones_col, ones_d[:, :])

    # resident output staging (DVE-written, DMA'd out once at the end)
    w_all = const.tile([NT, BL, P], F32, name="w_all")
    ctx_all = const.tile([1, BL, H], F32, name="ctx_all")
    s_all = const.tile([1, BL], F32, name="s_all")

    # warmups: absorb every const-DMA queue semaphore into the consuming
    # engine's observed clock (walrus allows only ONE sync-wait per
    # instruction, so no later instruction may need two fresh semaphores).
    warm_ps = sm_ps.tile([1, P], F32, name="warm_ps", tag="smA")
    nc.tensor.matmul(warm_ps[:, 0:P], ident[:, 0:1], ident, start=True, stop=True)
    for wi, wsrc in enumerate((wencT[:, 0, 0:1], v_col, ones_col)):
        nc.tensor.matmul(warm_ps[:, wi:wi + 1], wsrc, ident[:, 0:1],
                         start=True, stop=True)
    warm_sb = sb.tile([A, BL], F32, name="warm_sb", tag="warm", bufs=1)
    nc.scalar.copy(warm_sb, dec_proj)

    tr_alloc = 0  # tr-pool alloc counter -> psum slot = tr_alloc % 4

    # ---------------- main loop over local batches ----------------
    for b in range(BL):
        enc_b = encp.tile([P, NT, H], F32, name="enc_b", tag="enc_b")
        for j in range(NT):
            nc.sync.dma_start(enc_b[:, j, :], enc[b, ts(j, P), :])

        # scores in transposed layout: [t128-part, j]
        scb = sc_ps.tile([P, NT], F32, name="scb", tag="scb")

        for t in range(NTC):
            encT_tiles = []
            for c in range(HC):
                trp = tr_ps.tile([P, TCH], F32, name="trp", tag="tr")
                slot = tr_alloc % 4
                tr_alloc += 1
                for jj in range(JPC):
                    j = t * JPC + jj
                    nc.tensor.transpose(
                        _bc(trp[:, ts(jj, P)], TR_DT),
                        _bc(enc_b[:, j, ts(c, P)], TR_DT),
                        _bc(ident, TR_DT),
                    )
                encT = sb.tile([P, TCH], F32, name="encT", tag="encT", bufs=4)
                # evac engine fixed per psum slot -> slot-release waits are
                # subsumed by the previous proj matmul's wait on that engine
                if slot < ACT_SLOTS:
                    nc.scalar.copy(encT, trp)
                else:
                    nc.vector.tensor_copy(encT, trp)
                encT_tiles.append(encT)

            pj = pj_ps.tile([A, TCH], F32, name="pj", tag="pj")
            for c in range(HC):
                nc.tensor.matmul(pj, _bc(wencT[:, c, :], MM_DT),
                                 _bc(encT_tiles[c], MM_DT),
                                 start=(c == 0), stop=(c == HC - 1))

            tanh_sb = sb.tile([A, TCH], F32, name="tanh_sb", tag="tanh", bufs=3)
            nc.scalar.activation(tanh_sb, pj, Tanh, bias=dec_proj[:, b:b + 1])

            for jj in range(JPC):
                j = t * JPC + jj
                nc.tensor.matmul(scb[:, j:j + 1],
                                 _bc(tanh_sb[:, ts(jj, P)], MM_DT),
                                 _bc(v_col, MM_DT), start=True, stop=True)

        # -------- softmax pieces (normalization happens on the host) -----
        exp_sb = sb.tile([P, NT], F32, name="exp_sb", tag="exp", bufs=2)
        sums = sb.tile([P, 1], F32, name="sums", tag="sums", bufs=2)
        nc.scalar.activation(exp_sb, scb, Exp, accum_out=sums)
        tot_ps = sm_ps.tile([1, 1], F32, name="tot_ps", tag="smA")
        nc.tensor.matmul(tot_ps, sums, ones_col, start=True, stop=True)
        nc.vector.tensor_copy(s_all[:, b:b + 1], tot_ps)

        # unnormalized weight rows: transpose exp [128,16] -> [16,128]
        wrow_ps = sc_ps.tile([BL, P], F32, name="wrow_ps", tag="scb")
        nc.tensor.transpose(wrow_ps, exp_sb, ident)
        nc.vector.tensor_copy(w_all[:, b, :], wrow_ps)

        # context: ctx[1, 512] = sum_j exp[:, j].T @ enc_b[:, j, :]
        ctx_ps = sc_ps.tile([1, H], F32, name="ctx_ps", tag="scb")
        for j in range(NT):
            nc.tensor.matmul(ctx_ps, _bc(exp_sb[:, j:j + 1], CTX_DT),
                             _bc(enc_b[:, j, :], CTX_DT),
                             start=(j == 0), stop=(j == NT - 1))
        nc.vector.tensor_copy(ctx_all[:, b, :], ctx_ps)

    # ---------------- final output DMAs ----------------
    nc.sync.dma_start(wts_out[:, :].rearrange("b (j q) -> j b q", q=P), w_all)
    nc.sync.dma_start(ctx_out[:, :].rearrange("b h -> (b h)")[None, :],
                      ctx_all.rearrange("o b h -> o (b h)"))
    nc.sync.dma_start(s_out[:, :], s_all)


def build_kernel_bf16(ctx: ExitStack, tc: tile.TileContext, enc, wencT_d,
                      dec_proj_d, v_col_d, ones_d, ident_d, wts_out, ctx_out,
                      s_out):
    """bf16 data path, no DRAM bounce: enc fp32 is cast to bf16 on DVE;
    128x128 PE transpose-mode matmuls (1 cycle/row in bf16) build encT in
    PSUM (bf16), evacuated to SBUF by ACT/DVE (bf16 = faster copies);
    proj/scores/ctx matmuls all run in bf16 (1 cycle/row vs 4 for fp32).
    Trades ~2e-3 relative error for the fp32 double-pass + fp32-transpose
    PE time.
    """
    nc = tc.nc
    BF16 = mybir.dt.float16 if os.environ.get("BAH_16", "fp16") == "fp16" \
        else mybir.dt.bfloat16
    Tanh = mybir.ActivationFunctionType.Tanh
    Exp = mybir.ActivationFunctionType.Exp

    const = ctx.enter_context(tc.tile_pool(name="const", bufs=1))
    sb = ctx.enter_context(tc.tile_pool(name="sb", bufs=2))
    encp = ctx.enter_context(tc.tile_pool(name="encp", bufs=2))
    tr_ps = ctx.enter_context(tc.tile_pool(name="tr_ps", bufs=4, space="PSUM"))
    pj_ps = ctx.enter_context(tc.tile_pool(name="pj_ps", bufs=1, space="PSUM"))
    sc_ps = ctx.enter_context(tc.tile_pool(name="sc_ps", bufs=2, space="PSUM"))
    sm_ps = ctx.enter_context(tc.tile_pool(name="sm_ps", bufs=1, space="PSUM"))

    # constants (host-precomputed)
    identf = const.tile([P, P], F32, name="identf")
    nc.sync.dma_start(identf, ident_d[:, :])
    ident = const.tile([P, P], BF16, name="ident")
    nc.vector.tensor_copy(ident, identf)
    wencT = const.tile([P, HC, A], BF16, name="wencT")
    nc.sync.dma_start(wencT, wencT_d[:, :].rearrange("p (c a) -> p c a", a=A))
    dec_proj = const.tile([A, BL], F32, name="dec_proj")
    nc.sync.dma_start(dec_proj, dec_proj_d[:, :])
    v_colf = const.tile([P, 1], F32, name="v_colf")
    nc.sync.dma_start(v_colf, v_col_d[:, :])
    v_col = const.tile([P, 1], BF16, name="v_col")
    nc.vector.tensor_copy(v_col, v_colf)
    ones_col = const.tile([P, 1], F32, name="ones_col")
    nc.sync.dma_start(ones_col, ones_d[:, :])

    # resident output staging
    w_all = const.tile([NT, BL, P], F32, name="w_all")
    ctx_all = const.tile([1, BL, H], F32, name="ctx_all")
    s_all = const.tile([1, BL], F32, name="s_all")

    # warmups: absorb const-DMA queue sems into consuming engines' clocks
    warm_ps = sm_ps.tile([1, P], F32, name="warm_ps", tag="smA")
    nc.tensor.matmul(warm_ps, ones_col, identf, start=True, stop=True)
    nc.tensor.matmul(warm_ps[:, 0:1], wencT[:, 0, 0:1], v_col, start=True,
                     stop=True)
    warm_sb = sb.tile([A, BL], F32, name="warm_sb", tag="warm", bufs=1)
    nc.scalar.copy(warm_sb, dec_proj)

    NLD = 8          # enc fp32 loads per batch ([128, 2, 512] each)
    JPL = NT // NLD  # 2 j-tiles per load

    GRP = 4  # context col-tiling group size
    tr_alloc = 0
    exp16s = []
    enc16s = []
    for b in range(BL):
        enc16 = encp.tile([P, NT, H], BF16, name="enc16", tag="enc16",
                          bufs=GRP + 2)

        # load fp32, cast to 16-bit on DVE
        for l in range(NLD):
            enc_f = sb.tile([P, JPL, H], F32, name="enc_f", tag="encf", bufs=4)
            nc.sync.dma_start(enc_f, enc[b, ts(l, JPL * P), :]
                              .rearrange("(j p) h -> p j h", p=P))
            nc.vector.tensor_copy(enc16[:, ts(l, JPL), :], enc_f)

        scb = sc_ps.tile([P, NT], F32, name="scb", tag="scb")

        for t in range(NTC):
            encT_tiles = []
            for c in range(HC):
                trp = tr_ps.tile([P, TCH], BF16, name="trp", tag="tr")
                slot = tr_alloc % 4
                tr_alloc += 1
                for jj in range(JPC):
                    j = t * JPC + jj
                    nc.tensor.transpose(trp[:, ts(jj, P)],
                                        enc16[:, j, ts(c, P)], ident)
                encT = sb.tile([P, TCH], BF16, name="encT", tag="encT", bufs=4)
                if slot < ACT_SLOTS:
                    nc.scalar.copy(encT, trp)
                else:
                    nc.vector.tensor_copy(encT, trp)
                encT_tiles.append(encT)

            pj = pj_ps.tile([A, TCH], F32, name="pj", tag="pj")
            for c in range(HC):
                nc.tensor.matmul(pj, wencT[:, c, :], encT_tiles[c],
                                 start=(c == 0), stop=(c == HC - 1))
            tanh16 = sb.tile([A, TCH], BF16, name="tanh16", tag="tanh", bufs=3)
            nc.scalar.activation(tanh16, pj, Tanh, bias=dec_proj[:, b:b + 1])
            for jj in range(JPC):
                j = t * JPC + jj
                nc.tensor.matmul(scb[:, j:j + 1], tanh16[:, ts(jj, P)],
                                 v_col, start=True, stop=True)

        # softmax pieces (normalization on host)
        exp_sb = sb.tile([P, NT], F32, name="exp_sb", tag="exp", bufs=2)
        sums = sb.tile([P, 1], F32, name="sums", tag="sums", bufs=2)
        nc.scalar.activation(exp_sb, scb, Exp, accum_out=sums)
        exp16 = sb.tile([P, NT], BF16, name="exp16", tag="exp16", bufs=GRP + 2)
        nc.gpsimd.tensor_copy(exp16, exp_sb)
        exp16s.append(exp16)
        enc16s.append(enc16)
        tot_ps = sm_ps.tile([1, 1], F32, name="tot_ps", tag="smA")
        nc.tensor.matmul(tot_ps, sums, ones_col, start=True, stop=True)
        # critical-path psum evacuations go to ACT: DVE's queue runs ~10us
        # behind (bulk casts of future batches), and these copies gate psum
        # slot reuse for the next batch's scores/ctx matmuls
        nc.scalar.copy(s_all[:, b:b + 1], tot_ps)

        wrow_ps = sc_ps.tile([BL, P], F32, name="wrow_ps", tag="scb")
        nc.tensor.transpose(wrow_ps, exp_sb, identf)
        nc.scalar.copy(w_all[:, b, :], wrow_ps)

        # context for a group of GRP batches at once: 4 concurrent M=1
        # matmuls in distinct 32-column groups of the PE array
        if b % GRP == GRP - 1:
            ctx_ps = sc_ps.tile([P, H], F32, name="ctx_ps", tag="scb")
            for j in range(NT):
                for gi in range(GRP):
                    nc.tensor.matmul(
                        ctx_ps[32 * gi:32 * gi + 1, :],
                        exp16s[gi][:, j:j + 1], enc16s[gi][:, j, :],
                        start=(j == 0), stop=(j == NT - 1),
                        tile_position=(0, 32 * gi))
            for gi in range(GRP):
                nc.scalar.copy(ctx_all[:, b - GRP + 1 + gi, :],
                               ctx_ps[32 * gi:32 * gi + 1, :])
            exp16s = []
            enc16s = []

    nc.sync.dma_start(wts_out[:, :].rearrange("b (j q) -> j b q", q=P), w_all)
    nc.sync.dma_start(ctx_out[:, :].rearrange("b h -> (b h)")[None, :],
                      ctx_all.rearrange("o b h -> o (b h)"))
    nc.sync.dma_start(s_out[:, :], s_all)


def _split_multi_waits(nc):
    """Walrus in this toolchain allows exactly ONE sync-wait per engine
    instruction. Tile sometimes emits more (slot-release + data deps on
    different semaphores). Hoist extra waits onto injected same-engine
    NoOps placed immediately before the instruction — semantically
    identical (engine streams are in-order), costs ~a sequencer dispatch.
    """
    n_split = 0
    for fn in nc.m.functions:
        for blk in fn.blocks:
            ins_list = blk.instructions
            if not any(i.sync_info and i.sync_info.on_wait and
                       len(i.sync_info.on_wait) > 1 for i in ins_list):
                continue
            out = []
            for inst in ins_list:
                si = inst.sync_info
                waits = list(si.on_wait) if si is not None and si.on_wait else []
                if len(waits) > 1:
                    # same-semaphore duplicates: keep only the max tick
                    bysem = {}
                    for w in waits:
                        k = (w.sync_type, w.id)
                        prev = bysem.get(k)
                        if prev is None or (w.wait_value or 0) > (prev.wait_value or 0):
                            bysem[k] = w
                    waits = list(bysem.values())
                if len(waits) > 1:
                    for w in waits[:-1]:
                        nop = mybir.InstNoOp(name=f"{inst.name}-ws{n_split}",
                                             engine=inst.engine, ins=[], outs=[])
                        nop.sync_info = mybir.SyncInfo(on_wait=[w], on_update=[])
                        out.append(nop)
                        n_split += 1
                    si.on_wait = [waits[-1]]
                out.append(inst)
            ins_list[:] = out
    return n_split


MODE = os.environ.get("BAH_MODE", "bf16")  # "bf16" | "f32"


def build_nc():
    nc = bass.Bass(trn_type="TRN2")
    if MODE == "bf16":
        wdt = (mybir.dt.float16 if os.environ.get("BAH_16", "fp16") == "fp16"
               else mybir.dt.bfloat16)
    else:
        wdt = F32
    enc = nc.dram_tensor("enc", [BL, T, H], F32, kind="ExternalInput")
    wencT_d = nc.dram_tensor("wencT", [P, HC * A], wdt, kind="ExternalInput")
    dec_proj_d = nc.dram_tensor("dec_projT", [A, BL], F32, kind="ExternalInput")
    v_col_d = nc.dram_tensor("v_col", [P, 1], F32, kind="ExternalInput")
    ones_d = nc.dram_tensor("ones_col", [P, 1], F32, kind="ExternalInput")
    ident_d = nc.dram_tensor("ident", [P, P], F32, kind="ExternalInput")
    wts_out = nc.dram_tensor("wts_raw", [BL, T], F32, kind="ExternalOutput")
    ctx_out = nc.dram_tensor("ctx_raw", [BL, H], F32, kind="ExternalOutput")
    s_out = nc.dram_tensor("s_out", [1, BL], F32, kind="ExternalOutput")

    build = build_kernel_bf16 if MODE == "bf16" else build_kernel
    with tile.TileContext(nc) as tc:
        with ExitStack() as ctx:
            build(ctx, tc, enc, wencT_d, dec_proj_d, v_col_d, ones_d,
                  ident_d, wts_out, ctx_out, s_out)
    _split_multi_waits(nc)
    return nc


_CACHE = {}


def _get_nc():
    if "nc" not in _CACHE:
        _CACHE["nc"] = build_nc()
    return _CACHE["nc"]


def _in_maps(encoder_outputs, decoder_hidden, W_enc, W_dec, v_w):
    enc = np.ascontiguousarray(np.asarray(encoder_outputs, dtype=np.float32))
    dec = np.ascontiguousarray(np.asarray(decoder_hidden, dtype=np.float32))
    we = np.asarray(W_enc, dtype=np.float32)
    wd = np.asarray(W_dec, dtype=np.float32)
    vw = np.asarray(v_w, dtype=np.float32)

    # host-derived constants (replicated)
    wencT = np.ascontiguousarray(
        we.T.reshape(HC, P, A).transpose(1, 0, 2).reshape(P, HC * A))
    v_col = np.ascontiguousarray(vw.reshape(A, 1))
    if MODE == "bf16":
        if os.environ.get("BAH_16", "fp16") == "fp16":
            wencT = wencT.astype(np.float16)
        else:
            import ml_dtypes
            wencT = wencT.astype(ml_dtypes.bfloat16)
    ones = np.ones((P, 1), dtype=np.float32)
    ident = np.eye(P, dtype=np.float32)

    maps = []
    for i in range(NCORES):
        sl = slice(i * BL, (i + 1) * BL)
        dec_projT = np.ascontiguousarray(wd @ dec[sl].T)  # [A, BL]
        maps.append({
            "enc": np.ascontiguousarray(enc[sl]),
            "wencT": wencT, "dec_projT": dec_projT, "v_col": v_col,
            "ones_col": ones, "ident": ident,
        })
    return maps


def run(encoder_outputs, decoder_hidden, W_enc, W_dec, v_w, **run_kwargs):
    """Run on 8 cores; returns (BassKernelResults, context, weights)."""
    nc = _get_nc()
    maps = _in_maps(encoder_outputs, decoder_hidden, W_enc, W_dec, v_w)
    res = bass_utils.run_bass_kernel_spmd(nc, maps, core_ids=list(range(NCORES)),
                                          **run_kwargs)
    ctxs = []
    wtss = []
    for r in res.results:
        s = r["s_out"].reshape(BL, 1)  # per-batch exp sums
        ctxs.append(r["ctx_raw"] / s)
        wtss.append(r["wts_raw"] / s)
    return res, np.concatenate(ctxs, axis=0), np.concatenate(wtss, axis=0)


def kernel(encoder_outputs, decoder_hidden, W_enc, W_dec, v_w):
    _, ctx, wts = run(encoder_outputs, decoder_hidden, W_enc, W_dec, v_w)
    return ctx, wts


if __name__ == "__main__":
    nc = _get_nc()
    print("built ok")
    if os.environ.get("BAH_COMPILE", "0") == "1":
        import tempfile
        d = tempfile.mkdtemp(prefix="bahcompile")
        print("compiling to", d)
        print("NEFF:", bass_utils.compile_bass_kernel(nc, d))
